# revision 2
# baseline (speedup 1.0000x reference)
"""Trainium2 Bass kernel for DigitConvolutionalModel.

Model: out = relu(conv3x3(x) @ w1.T + b1) @ w2.T + b2

The 3x3 valid cross-correlation is linear in x, so it is folded into the
first linear layer on the host: W1_eff[h, :] = conv-smeared w1[h, :],
giving a plain 2-layer MLP:

    out = relu(x @ W1_eff.T + b1) @ w2.T + b2      x: [65536, 784]

Sharding: pure data parallelism — batch split 8 ways (8192 rows/core),
weights replicated. Per core the kernel computes out.T [10, 8192] with
batch on the matmul free dim and features on partitions; x is transposed
and cast to bf16 on the host during sharding, so all device DMAs are
natural-layout. Contraction dims are zero-padded to K=128 multiples
(784 -> 6x128 + one remainder block padded on-chip; hid 100 -> 128) so
every matmul contracts over full 128 partitions.
"""

import sys

import numpy as np

if "/opt/trn_rl_repo" not in sys.path:
    sys.path.insert(0, "/opt/trn_rl_repo")

import ml_dtypes

B = 65536
IMG = 28
KSZ = 3
OUT_HW = IMG - KSZ + 1  # 26
FLAT = OUT_HW * OUT_HW  # 676
HID = 100
NCLS = 10
FEAT = IMG * IMG  # 784

N_CORES = 8
BPC = B // N_CORES  # 8192 batch rows per core
KMAIN = 6  # full 128-row feature chunks (768 rows)
KC = KMAIN + 1  # + remainder chunk (rows 768..783, zero-padded)
FPAD = KC * 128  # 896
HPAD = 128  # hidden dim padded 100 -> 128
NB = 512  # batch tile (matmul free dim)
NCHUNK = BPC // NB  # 16

_BF16 = ml_dtypes.bfloat16
_CACHE = {}


def _build_module():
    import concourse.tile as tile
    from concourse import bacc, mybir

    nc = bacc.Bacc(
        "TRN2", target_bir_lowering=False, debug=False, num_devices=N_CORES
    )
    xt = nc.dram_tensor("xt", [FEAT, BPC], mybir.dt.bfloat16, kind="ExternalInput").ap()
    w1t = nc.dram_tensor(
        "w1t", [FPAD, HPAD], mybir.dt.bfloat16, kind="ExternalInput"
    ).ap()
    b1 = nc.dram_tensor("b1", [HPAD], mybir.dt.float32, kind="ExternalInput").ap()
    w2t = nc.dram_tensor(
        "w2t", [HPAD, NCLS], mybir.dt.bfloat16, kind="ExternalInput"
    ).ap()
    b2 = nc.dram_tensor("b2", [NCLS], mybir.dt.float32, kind="ExternalInput").ap()
    outt = nc.dram_tensor(
        "outt", [NCLS, BPC], mybir.dt.float32, kind="ExternalOutput"
    ).ap()

    relu = mybir.ActivationFunctionType.Relu

    with tile.TileContext(nc) as tc:
        with (
            tc.tile_pool(name="const", bufs=1) as constp,
            tc.tile_pool(name="xin", bufs=4) as xp,
            tc.tile_pool(name="h1", bufs=3) as hp,
            tc.tile_pool(name="ob", bufs=3) as op,
            tc.tile_pool(name="ps1", bufs=4, space="PSUM") as pp1,
            tc.tile_pool(name="ps2", bufs=2, space="PSUM") as pp2,
        ):
            # Replicated weights/biases, loaded once.
            W1 = constp.tile([128, KC, HPAD], mybir.dt.bfloat16)
            nc.sync.dma_start(W1[:], w1t.rearrange("(c p) m -> p c m", p=128))
            W2 = constp.tile([HPAD, NCLS], mybir.dt.bfloat16)
            nc.sync.dma_start(W2[:], w2t)
            B1 = constp.tile([HPAD, 1], mybir.dt.float32)
            nc.sync.dma_start(B1[:], b1.rearrange("(p o) -> p o", o=1))
            B2 = constp.tile([NCLS, 1], mybir.dt.float32)
            nc.sync.dma_start(B2[:], b2.rearrange("(p o) -> p o", o=1))

            # Remainder feature rows (768..783) for the whole batch shard,
            # resident in SBUF; rows 16..127 zeroed so the K=128 contraction
            # over the zero-padded w1t rows sees no garbage.
            XR = constp.tile([128, BPC], mybir.dt.bfloat16)
            nc.vector.memset(XR[:, :], 0.0)
            nc.sync.dma_start(XR[:16, :], xt[KMAIN * 128 : FEAT, :])

            xmain = xt[: KMAIN * 128].rearrange("(c p) b -> p c b", p=128)

            for n in range(NCHUNK):
                nsl = slice(n * NB, (n + 1) * NB)
                xa = xp.tile([128, KMAIN, NB], mybir.dt.bfloat16)
                nc.sync.dma_start(xa[:], xmain[:, :, nsl])

                ps1 = pp1.tile([128, NB], mybir.dt.float32)
                for c in range(KMAIN):
                    nc.tensor.matmul(
                        ps1[:], W1[:, c, :], xa[:, c, :], start=(c == 0), stop=False
                    )
                nc.tensor.matmul(
                    ps1[:], W1[:, KMAIN, :], XR[:, nsl], start=False, stop=True
                )

                h1 = hp.tile([128, NB], mybir.dt.bfloat16)
                nc.scalar.activation(h1[:], ps1[:], relu, bias=B1[:])

                ps2 = pp2.tile([NCLS, NB], mybir.dt.float32)
                nc.tensor.matmul(ps2[:], W2[:], h1[:], start=True, stop=True)

                ob = op.tile([NCLS, NB], mybir.dt.float32)
                nc.vector.tensor_scalar_add(ob[:], ps2[:], B2[:])
                nc.sync.dma_start(outt[:, nsl], ob[:])

    nc.compile()
    return nc


def _get_module():
    nc = _CACHE.get("nc")
    if nc is None:
        nc = _build_module()
        _CACHE["nc"] = nc
    return nc


def _prepare_inputs(x, conv_w, w1, b1, w2, b2):
    x = np.asarray(x, dtype=np.float32)
    conv_w = np.asarray(conv_w, dtype=np.float32)
    w1 = np.asarray(w1, dtype=np.float32)
    b1 = np.asarray(b1, dtype=np.float32)
    w2 = np.asarray(w2, dtype=np.float32)
    b2 = np.asarray(b2, dtype=np.float32)

    # Fold the 3x3 cross-correlation into w1: for each hidden unit h,
    # W1_eff[h, p, q] = sum over (i, j, di, dj) with (p, q) = (i+di, j+dj)
    # of w1[h, i*26+j] * conv_w[di, dj].
    w1r = w1.reshape(HID, OUT_HW, OUT_HW)
    w1_eff = np.zeros((HID, IMG, IMG), np.float32)
    for di in range(KSZ):
        for dj in range(KSZ):
            w1_eff[:, di : di + OUT_HW, dj : dj + OUT_HW] += conv_w[di, dj] * w1r

    w1t_pad = np.zeros((FPAD, HPAD), _BF16)
    w1t_pad[:FEAT, :HID] = w1_eff.reshape(HID, FEAT).T.astype(_BF16)
    b1_pad = np.zeros(HPAD, np.float32)
    b1_pad[:HID] = b1
    w2t_pad = np.zeros((HPAD, NCLS), _BF16)
    w2t_pad[:HID, :] = w2.T.astype(_BF16)

    # Per-core transposed bf16 x shards: [core][feat, batch].
    xs = x.reshape(N_CORES, BPC, FEAT).astype(_BF16)
    in_maps = []
    for i in range(N_CORES):
        in_maps.append(
            {
                "xt": np.ascontiguousarray(xs[i].T),
                "w1t": w1t_pad,
                "b1": b1_pad,
                "w2t": w2t_pad,
                "b2": b2,
            }
        )
    return in_maps


def _run_device(in_maps, trace=False, trace_cores=None):
    from concourse.bass_utils import run_bass_kernel_spmd

    nc = _get_module()
    return run_bass_kernel_spmd(
        nc,
        in_maps,
        core_ids=list(range(N_CORES)),
        trace=trace,
        trace_cores=trace_cores,
    )


def kernel(x, conv_w, w1, b1, w2, b2):
    in_maps = _prepare_inputs(x, conv_w, w1, b1, w2, b2)
    res = _run_device(in_maps)
    out = np.empty((B, NCLS), np.float32)
    for i in range(N_CORES):
        out[i * BPC : (i + 1) * BPC] = res.results[i]["outt"].T
    return out


# revision 4
# speedup vs baseline: 1.0885x; 1.0885x over previous
"""Trainium2 Bass kernel for DigitConvolutionalModel.

Model: out = relu(conv3x3(x) @ w1.T + b1) @ w2.T + b2

The 3x3 valid cross-correlation is linear in x, so it is folded into the
first linear layer on the host: W1_eff[h, :] = conv-smeared w1[h, :],
giving a plain 2-layer MLP:

    out = relu(x @ W1_eff.T + b1) @ w2.T + b2      x: [65536, 784]

Sharding: pure data parallelism — batch split 8 ways (8192 rows/core),
weights replicated. Per core the kernel computes out.T [10, 8192] with
batch on the matmul free dim and features on partitions. The host casts
x to bf16 and lays it out in the exact blocked SBUF tile order
([chunk][partition=feat%128][featchunk][batch]) so every device DMA is
one fully-contiguous transfer.

Other tricks:
 - hidden dim padded 100 -> 128 with zero weight columns; b1_pad[100]=1
   makes h1 row 100 == relu(0+1) == 1.0, and W2T row 100 = b2, so the
   second-layer bias is folded into the second matmul for free.
 - feature remainder (rows 768..783) handled by a K=16 matmul against a
   resident [16, 8192] tile (verified exact on HW), no padding needed.
"""

import sys

import numpy as np

if "/opt/trn_rl_repo" not in sys.path:
    sys.path.insert(0, "/opt/trn_rl_repo")

import ml_dtypes

B = 65536
IMG = 28
KSZ = 3
OUT_HW = IMG - KSZ + 1  # 26
FLAT = OUT_HW * OUT_HW  # 676
HID = 100
NCLS = 10
FEAT = IMG * IMG  # 784

N_CORES = 8
BPC = B // N_CORES  # 8192 batch rows per core
KMAIN = 6  # full 128-row feature chunks (768 rows)
KREM = FEAT - KMAIN * 128  # 16 remainder feature rows
HPAD = 128  # hidden dim padded 100 -> 128 (row 100 = bias carrier)
NB = 1024  # batch rows per x-DMA chunk
NSUB = NB // 512  # 512-wide matmul subtiles per chunk
NCHUNK = BPC // NB  # 8

_BF16 = ml_dtypes.bfloat16
_CACHE = {}

X_BUFS = 4
PS1_BUFS = 3
PS2_BUFS = 2


def _build_module():
    import concourse.tile as tile
    from concourse import bacc, mybir

    nc = bacc.Bacc(
        "TRN2", target_bir_lowering=False, debug=False, num_devices=N_CORES
    )
    xm = nc.dram_tensor(
        "xm", [NCHUNK, 128, KMAIN * NB], mybir.dt.bfloat16, kind="ExternalInput"
    ).ap()
    xr = nc.dram_tensor("xr", [KREM, BPC], mybir.dt.bfloat16, kind="ExternalInput").ap()
    w1t = nc.dram_tensor(
        "w1t", [FEAT, HPAD], mybir.dt.bfloat16, kind="ExternalInput"
    ).ap()
    b1 = nc.dram_tensor("b1", [HPAD], mybir.dt.float32, kind="ExternalInput").ap()
    w2t = nc.dram_tensor(
        "w2t", [HPAD, NCLS], mybir.dt.bfloat16, kind="ExternalInput"
    ).ap()
    outt = nc.dram_tensor(
        "outt", [NCLS, BPC], mybir.dt.float32, kind="ExternalOutput"
    ).ap()

    relu = mybir.ActivationFunctionType.Relu

    with tile.TileContext(nc) as tc:
        with (
            tc.tile_pool(name="const", bufs=1) as constp,
            tc.tile_pool(name="xin", bufs=X_BUFS) as xp,
            tc.tile_pool(name="h1", bufs=3) as hp,
            tc.tile_pool(name="ob", bufs=3) as op,
            tc.tile_pool(name="ps1", bufs=PS1_BUFS, space="PSUM") as pp1,
            tc.tile_pool(name="ps2", bufs=PS2_BUFS, space="PSUM") as pp2,
        ):
            # Replicated weights/biases, loaded once.
            W1 = constp.tile([128, KMAIN, HPAD], mybir.dt.bfloat16)
            nc.sync.dma_start(
                W1[:], w1t[: KMAIN * 128].rearrange("(c p) m -> p c m", p=128)
            )
            W1R = constp.tile([KREM, HPAD], mybir.dt.bfloat16)
            nc.sync.dma_start(W1R[:], w1t[KMAIN * 128 : FEAT])
            W2 = constp.tile([HPAD, NCLS], mybir.dt.bfloat16)
            nc.sync.dma_start(W2[:], w2t)
            B1 = constp.tile([HPAD, 1], mybir.dt.float32)
            nc.sync.dma_start(B1[:], b1.rearrange("(p o) -> p o", o=1))

            # Remainder feature rows resident for the whole batch shard.
            XR = constp.tile([KREM, BPC], mybir.dt.bfloat16)
            nc.sync.dma_start(XR[:], xr[:])

            for n in range(NCHUNK):
                xa = xp.tile([128, KMAIN, NB], mybir.dt.bfloat16)
                nc.sync.dma_start(
                    xa[:], xm[n].rearrange("p (c b) -> p c b", c=KMAIN)
                )

                ps1 = pp1.tile([128, NB], mybir.dt.float32)
                for c in range(KMAIN):
                    for s in range(NSUB):
                        ssl = slice(s * 512, (s + 1) * 512)
                        nc.tensor.matmul(
                            ps1[:, ssl],
                            W1[:, c, :],
                            xa[:, c, ssl],
                            start=(c == 0),
                            stop=False,
                        )
                for s in range(NSUB):
                    ssl = slice(s * 512, (s + 1) * 512)
                    bsl = slice(n * NB + s * 512, n * NB + (s + 1) * 512)
                    nc.tensor.matmul(
                        ps1[:, ssl], W1R[:], XR[:, bsl], start=False, stop=True
                    )

                h1 = hp.tile([128, NB], mybir.dt.bfloat16)
                nc.scalar.activation(h1[:], ps1[:], relu, bias=B1[:])

                ob = op.tile([NCLS, NB], mybir.dt.float32)
                for s in range(NSUB):
                    ssl = slice(s * 512, (s + 1) * 512)
                    ps2 = pp2.tile([NCLS, 512], mybir.dt.float32)
                    nc.tensor.matmul(ps2[:], W2[:], h1[:, ssl], start=True, stop=True)
                    nc.vector.tensor_copy(ob[:, ssl], ps2[:])
                nc.sync.dma_start(outt[:, n * NB : (n + 1) * NB], ob[:])

    nc.compile()
    return nc


def _get_module():
    nc = _CACHE.get("nc")
    if nc is None:
        nc = _build_module()
        _CACHE["nc"] = nc
    return nc


def _prepare_inputs(x, conv_w, w1, b1, w2, b2):
    x = np.asarray(x, dtype=np.float32)
    conv_w = np.asarray(conv_w, dtype=np.float32)
    w1 = np.asarray(w1, dtype=np.float32)
    b1 = np.asarray(b1, dtype=np.float32)
    w2 = np.asarray(w2, dtype=np.float32)
    b2 = np.asarray(b2, dtype=np.float32)

    # Fold the 3x3 cross-correlation into w1: for each hidden unit h,
    # W1_eff[h, p, q] = sum over (i, j, di, dj) with (p, q) = (i+di, j+dj)
    # of w1[h, i*26+j] * conv_w[di, dj].
    w1r = w1.reshape(HID, OUT_HW, OUT_HW)
    w1_eff = np.zeros((HID, IMG, IMG), np.float32)
    for di in range(KSZ):
        for dj in range(KSZ):
            w1_eff[:, di : di + OUT_HW, dj : dj + OUT_HW] += conv_w[di, dj] * w1r

    w1t_pad = np.zeros((FEAT, HPAD), _BF16)
    w1t_pad[:, :HID] = w1_eff.reshape(HID, FEAT).T.astype(_BF16)
    b1_pad = np.zeros(HPAD, np.float32)
    b1_pad[:HID] = b1
    b1_pad[HID] = 1.0  # h1 row 100 becomes relu(0+1) == 1: carries b2
    w2t_pad = np.zeros((HPAD, NCLS), _BF16)
    w2t_pad[:HID, :] = w2.T.astype(_BF16)
    w2t_pad[HID, :] = b2.astype(_BF16)

    # Per-core blocked bf16 x: xm[n, p, c*NB+b] = x[n*NB+b, c*128+p]
    xb = x.astype(_BF16)
    xcores = xb.reshape(N_CORES, NCHUNK, NB, FEAT)
    xm_all = np.ascontiguousarray(
        xcores[:, :, :, : KMAIN * 128]
        .reshape(N_CORES, NCHUNK, NB, KMAIN, 128)
        .transpose(0, 1, 4, 3, 2)
    ).reshape(N_CORES, NCHUNK, 128, KMAIN * NB)
    xr_all = np.ascontiguousarray(
        xb.reshape(N_CORES, BPC, FEAT)[:, :, KMAIN * 128 :].transpose(0, 2, 1)
    )

    in_maps = []
    for i in range(N_CORES):
        in_maps.append(
            {
                "xm": xm_all[i],
                "xr": xr_all[i],
                "w1t": w1t_pad,
                "b1": b1_pad,
                "w2t": w2t_pad,
            }
        )
    return in_maps


def _run_device(in_maps, trace=False, trace_cores=None):
    from concourse.bass_utils import run_bass_kernel_spmd

    nc = _get_module()
    return run_bass_kernel_spmd(
        nc,
        in_maps,
        core_ids=list(range(N_CORES)),
        trace=trace,
        trace_cores=trace_cores,
    )


def kernel(x, conv_w, w1, b1, w2, b2):
    in_maps = _prepare_inputs(x, conv_w, w1, b1, w2, b2)
    res = _run_device(in_maps)
    out = np.empty((B, NCLS), np.float32)
    for i in range(N_CORES):
        out[i * BPC : (i + 1) * BPC] = res.results[i]["outt"].T
    return out


# revision 8
# speedup vs baseline: 1.3835x; 1.2710x over previous
"""Trainium2 Bass kernel for DigitConvolutionalModel.

Model: out = relu(conv3x3(x) @ w1.T + b1) @ w2.T + b2

The 3x3 valid cross-correlation is linear in x, so it is folded into the
first linear layer on the host: W1_eff[h, :] = conv-smeared w1[h, :],
giving a plain 2-layer MLP:

    out = relu(x @ W1_eff.T + b1) @ w2.T + b2      x: [65536, 784]

Sharding: pure data parallelism — batch split 8 ways (8192 rows/core),
weights replicated. Per core the kernel computes out.T [10, 8192] with
batch on the matmul free dim and features on partitions. The host casts
x to bf16 and lays it out in the exact blocked SBUF tile order
([chunk][partition=feat%128][featchunk][batch]) so every device x DMA is
one fully-contiguous transfer that streams through the HWDGE queue FIFO.

Tricks:
 - hidden dim padded 100 -> 128 with zero weight columns; b1_pad[100]=1
   makes h1 row 100 == relu(0+1) == 1.0, and W2T row 100 = b2, so the
   second-layer bias is folded into the second matmul for free.
 - feature remainder (rows 768..783) handled by K=16 matmuls against a
   resident [128, 2048] tile holding batch groups of 2048 at 32-aligned
   partition offsets (0/32/64/96), keeping matmul base partitions legal.
 - weights/bias/remainder DMAs are issued from the GpSimd queue so the
   Sync HWDGE queue carries nothing but the x stream (strict FIFO).
"""

import sys

import numpy as np

if "/opt/trn_rl_repo" not in sys.path:
    sys.path.insert(0, "/opt/trn_rl_repo")

import ml_dtypes

B = 65536
IMG = 28
KSZ = 3
OUT_HW = IMG - KSZ + 1  # 26
FLAT = OUT_HW * OUT_HW  # 676
HID = 100
NCLS = 10
FEAT = IMG * IMG  # 784

N_CORES = 8
BPC = B // N_CORES  # 8192 batch rows per core
KMAIN = 6  # full 128-row feature chunks (768 rows)
KREM = FEAT - KMAIN * 128  # 16 remainder feature rows
HPAD = 128  # hidden dim padded 100 -> 128 (row 100 = bias carrier)
NB = 1024  # batch rows per chunk
NSUB = NB // 512  # 512-wide matmul subtiles per chunk
NCHUNK = BPC // NB  # 8
KHALF = KMAIN // 2  # k-blocks per half-chunk DMA
XR_GRP = 2048  # batch rows per 32-partition group in the XR tile

_BF16 = ml_dtypes.bfloat16
_CACHE = {}

X_BUFS = 6
PS1_BUFS = 3
PS2_BUFS = 2


def _build_module():
    import concourse.tile as tile
    from concourse import bacc, mybir

    nc = bacc.Bacc(
        "TRN2", target_bir_lowering=False, debug=False, num_devices=N_CORES
    )
    xm = nc.dram_tensor(
        "xm", [NCHUNK, 2, 128, KHALF * NB], mybir.dt.bfloat16, kind="ExternalInput"
    ).ap()
    xr = nc.dram_tensor(
        "xr", [128, XR_GRP], mybir.dt.bfloat16, kind="ExternalInput"
    ).ap()
    w1m = nc.dram_tensor(
        "w1m", [128, KMAIN * HPAD], mybir.dt.bfloat16, kind="ExternalInput"
    ).ap()
    w1r = nc.dram_tensor(
        "w1r", [128, HPAD], mybir.dt.bfloat16, kind="ExternalInput"
    ).ap()
    b1 = nc.dram_tensor("b1", [HPAD], mybir.dt.float32, kind="ExternalInput").ap()
    w2t = nc.dram_tensor(
        "w2t", [HPAD, NCLS], mybir.dt.bfloat16, kind="ExternalInput"
    ).ap()
    outt = nc.dram_tensor(
        "outt", [NCLS, BPC], mybir.dt.float32, kind="ExternalOutput"
    ).ap()

    relu = mybir.ActivationFunctionType.Relu

    with tile.TileContext(nc) as tc:
        with (
            tc.tile_pool(name="const", bufs=1) as constp,
            tc.tile_pool(name="xin", bufs=X_BUFS) as xp,
            tc.tile_pool(name="h1", bufs=3) as hp,
            tc.tile_pool(name="ob", bufs=3) as op,
            tc.tile_pool(name="ps1", bufs=PS1_BUFS, space="PSUM") as pp1,
            tc.tile_pool(name="ps2", bufs=PS2_BUFS, space="PSUM") as pp2,
        ):
            # Replicated weights/biases + resident remainder rows; all on the
            # GpSimd queue so the Sync HWDGE queue only carries x chunks.
            W1 = constp.tile([128, KMAIN, HPAD], mybir.dt.bfloat16)
            nc.gpsimd.dma_start(W1[:], w1m.rearrange("p (c m) -> p c m", c=KMAIN))
            # w1r replicated at partition offsets 0/32/64/96 so the K=16
            # remainder matmuls can match the XR group base partition.
            W1R = constp.tile([128, HPAD], mybir.dt.bfloat16)
            nc.gpsimd.dma_start(W1R[:], w1r[:])
            W2 = constp.tile([HPAD, NCLS], mybir.dt.bfloat16)
            nc.gpsimd.dma_start(W2[:], w2t[:])
            B1 = constp.tile([HPAD, 1], mybir.dt.float32)
            nc.gpsimd.dma_start(B1[:], b1.rearrange("(p o) -> p o", o=1))
            XR = constp.tile([128, XR_GRP], mybir.dt.bfloat16)
            nc.gpsimd.dma_start(XR[:], xr[:])

            for n in range(NCHUNK):
                xh = [None, None]
                for h in range(2):
                    xh[h] = xp.tile(
                        [128, KHALF, NB], mybir.dt.bfloat16, tag="xh", name=f"xh{h}"
                    )
                    nc.sync.dma_start(
                        xh[h][:], xm[n, h].rearrange("p (c b) -> p c b", c=KHALF)
                    )

                ps1 = pp1.tile([128, NB], mybir.dt.float32)
                for c in range(KMAIN):
                    for s in range(NSUB):
                        ssl = slice(s * 512, (s + 1) * 512)
                        nc.tensor.matmul(
                            ps1[:, ssl],
                            W1[:, c, :],
                            xh[c // KHALF][:, c % KHALF, ssl],
                            start=(c == 0),
                            stop=False,
                        )
                for s in range(NSUB):
                    ssl = slice(s * 512, (s + 1) * 512)
                    boff = n * NB + s * 512
                    g, coff = divmod(boff, XR_GRP)
                    nc.tensor.matmul(
                        ps1[:, ssl],
                        W1R[32 * g : 32 * g + KREM, :],
                        XR[32 * g : 32 * g + KREM, coff : coff + 512],
                        start=False,
                        stop=True,
                        tile_position=(32 * g, 0) if g == 3 else None,
                    )

                h1 = hp.tile([128, NB], mybir.dt.bfloat16)
                nc.scalar.activation(h1[:], ps1[:], relu, bias=B1[:])

                ob = op.tile([NCLS, NB], mybir.dt.float32)
                for s in range(NSUB):
                    ssl = slice(s * 512, (s + 1) * 512)
                    ps2 = pp2.tile([NCLS, 512], mybir.dt.float32)
                    nc.tensor.matmul(ps2[:], W2[:], h1[:, ssl], start=True, stop=True)
                    nc.vector.tensor_copy(ob[:, ssl], ps2[:])
                nc.gpsimd.dma_start(outt[:, n * NB : (n + 1) * NB], ob[:])

    nc.compile()
    return nc


def _get_module():
    nc = _CACHE.get("nc")
    if nc is None:
        nc = _build_module()
        _CACHE["nc"] = nc
    return nc


def _prepare_inputs(x, conv_w, w1, b1, w2, b2):
    x = np.asarray(x, dtype=np.float32)
    conv_w = np.asarray(conv_w, dtype=np.float32)
    w1 = np.asarray(w1, dtype=np.float32)
    b1 = np.asarray(b1, dtype=np.float32)
    w2 = np.asarray(w2, dtype=np.float32)
    b2 = np.asarray(b2, dtype=np.float32)

    # Fold the 3x3 cross-correlation into w1: for each hidden unit h,
    # W1_eff[h, p, q] = sum over (i, j, di, dj) with (p, q) = (i+di, j+dj)
    # of w1[h, i*26+j] * conv_w[di, dj].
    w1im = w1.reshape(HID, OUT_HW, OUT_HW)
    w1_eff = np.zeros((HID, IMG, IMG), np.float32)
    for di in range(KSZ):
        for dj in range(KSZ):
            w1_eff[:, di : di + OUT_HW, dj : dj + OUT_HW] += conv_w[di, dj] * w1im

    w1t_pad = np.zeros((FEAT, HPAD), _BF16)
    w1t_pad[:, :HID] = w1_eff.reshape(HID, FEAT).T.astype(_BF16)
    # blocked: w1m[p, c*HPAD + m] = w1t_pad[c*128 + p, m]
    w1m_host = np.ascontiguousarray(
        w1t_pad[: KMAIN * 128].reshape(KMAIN, 128, HPAD).transpose(1, 0, 2)
    ).reshape(128, KMAIN * HPAD)
    w1r_host = np.zeros((128, HPAD), _BF16)
    for g in range(4):
        w1r_host[32 * g : 32 * g + KREM] = w1t_pad[KMAIN * 128 : FEAT]

    b1_pad = np.zeros(HPAD, np.float32)
    b1_pad[:HID] = b1
    b1_pad[HID] = 1.0  # h1 row 100 becomes relu(0+1) == 1: carries b2
    w2t_pad = np.zeros((HPAD, NCLS), _BF16)
    w2t_pad[:HID, :] = w2.T.astype(_BF16)
    w2t_pad[HID, :] = b2.astype(_BF16)

    xb = x.astype(_BF16)
    # xm[n, h, p, c*NB+b] = x[n*NB+b, (h*KHALF+c)*128+p]
    xcores = xb.reshape(N_CORES, NCHUNK, NB, FEAT)
    xm_all = np.ascontiguousarray(
        xcores[:, :, :, : KMAIN * 128]
        .reshape(N_CORES, NCHUNK, NB, 2, KHALF, 128)
        .transpose(0, 1, 3, 5, 4, 2)
    ).reshape(N_CORES, NCHUNK, 2, 128, KHALF * NB)
    # xr: batch groups of XR_GRP at partition offsets 32g..32g+15
    n_grp = BPC // XR_GRP
    xr_all = np.zeros((N_CORES, 128, XR_GRP), _BF16)
    rem = xb.reshape(N_CORES, BPC, FEAT)[:, :, KMAIN * 128 :]  # [cores, BPC, 16]
    rem_g = rem.reshape(N_CORES, n_grp, XR_GRP, KREM).transpose(0, 1, 3, 2)
    for g in range(n_grp):
        xr_all[:, 32 * g : 32 * g + KREM, :] = rem_g[:, g]

    in_maps = []
    for i in range(N_CORES):
        in_maps.append(
            {
                "xm": xm_all[i],
                "xr": xr_all[i],
                "w1m": w1m_host,
                "w1r": w1r_host,
                "b1": b1_pad,
                "w2t": w2t_pad,
            }
        )
    return in_maps


def _run_device(in_maps, trace=False, trace_cores=None):
    from concourse.bass_utils import run_bass_kernel_spmd

    nc = _get_module()
    return run_bass_kernel_spmd(
        nc,
        in_maps,
        core_ids=list(range(N_CORES)),
        trace=trace,
        trace_cores=trace_cores,
    )


def kernel(x, conv_w, w1, b1, w2, b2):
    in_maps = _prepare_inputs(x, conv_w, w1, b1, w2, b2)
    res = _run_device(in_maps)
    out = np.empty((B, NCLS), np.float32)
    for i in range(N_CORES):
        out[i * BPC : (i + 1) * BPC] = res.results[i]["outt"].T
    return out


# revision 9
# speedup vs baseline: 1.4983x; 1.0830x over previous
"""Trainium2 Bass kernel for DigitConvolutionalModel (self-contained).

Model: out = relu(conv3x3(x) @ w1.T + b1) @ w2.T + b2, x: [65536, 784] f32.

Algorithm
---------
The 3x3 valid cross-correlation is linear in x, so it is folded into the
first linear layer on the host (W1_eff[h] = conv-smeared w1[h]), giving a
plain 2-layer MLP:  out = relu(x @ W1_eff.T + b1) @ w2.T + b2.

Sharding: pure data parallelism — batch split 8 ways (8192 rows/core),
weights replicated; no collectives. Per core the kernel computes
out.T [10, 8192] with batch on the matmul free dim and features on
partitions; the host casts x to bf16 (fp32 accumulate in PSUM, measured
rel err ~3e-3) and lays it out in the exact blocked SBUF tile order
([chunk][partition = feat%128][feat chunk][batch]), so every device x DMA
is one fully contiguous transfer streaming through the Sync HWDGE FIFO at
~420 GB/s. The kernel is HBM-bandwidth-bound (12.6 MB of x per core);
TensorE work (~27 us warm) hides completely under the DMA stream.

Device pipeline (hand-written bacc, ~20 semaphores, no Tile scheduler):
  Sync   : consts + x half-chunk stream (strict FIFO, nothing else queued)
  Tensor : L1(0) L1(1) L2(0) L1(2) L2(1) ... L1(7) L2(6) L2(7)
           L1(n) = 12 K=128 matmuls + 2 K=16 remainder matmuls -> ps1 ring
           L2(n) = 2 matmuls h1 @ W2 -> ps2 ring
  Scalar : relu(ps1 + b1) -> h1 bf16, plus output DMAs (own HWDGE queue),
           lagged two chunks so they stay off the critical path
  Vector : ps2 -> ob f32 copies (PSUM cannot be DMA'd directly)

Tricks:
 - hidden dim padded 100 -> 128 with zero weight columns; b1_pad[100] = 1
   makes h1 row 100 == relu(0+1) == 1.0 and W2T row 100 = b2, folding the
   second-layer bias into the second matmul for free.
 - feature remainder (rows 768..783) handled by K=16 matmuls against a
   resident [128, 2048] tile holding batch groups at 32-aligned partition
   offsets (matmul base partitions must be 32-aligned; the 96 group needs
   an explicit tile_position).
 - all small constants (blocked W1, replicated W1 remainder, W2T+b2 rows,
   b1) are byte-packed into one [128, 1816] uint8 tensor: one contiguous
   DMA, no tiny-packet head-of-queue blocking; device uses bitcast views.
 - per-DMA-target semaphores with at most one outstanding DMA each
   (concurrent DMA slice completions interleave across queues, so shared
   counting semaphores would be racy).
"""

import sys

import numpy as np

if "/opt/trn_rl_repo" not in sys.path:
    sys.path.insert(0, "/opt/trn_rl_repo")

import ml_dtypes

B = 65536
IMG = 28
KSZ = 3
OUT_HW = IMG - KSZ + 1  # 26
FLAT = OUT_HW * OUT_HW  # 676
HID = 100
NCLS = 10
FEAT = IMG * IMG  # 784

N_CORES = 8
BPC = B // N_CORES  # 8192 batch rows per core
KMAIN = 6  # full 128-row feature chunks (768 rows)
KREM = FEAT - KMAIN * 128  # 16 remainder feature rows
HPAD = 128  # hidden dim padded 100 -> 128 (row 100 = bias carrier)
NB = 1024  # batch rows per chunk
NSUB = NB // 512  # 512-wide matmul subtiles per chunk
NCHUNK = BPC // NB  # 8
KHALF = KMAIN // 2  # k-blocks per half-chunk DMA
XR_GRP = 2048  # batch rows per 32-partition group in the XR tile

NXBUF = 8  # x half-chunk slot ring
NPS1 = 3  # ps1 ring (2 PSUM banks each)
NPS2 = 2  # ps2 ring (1 bank each)
NH1 = 3
NOB = 3
CPK_BYTES = 1816  # packed const bytes per partition

_BF16 = ml_dtypes.bfloat16
_CACHE = {}


def _build_module():
    import contextlib

    from concourse import bacc, mybir

    nc = bacc.Bacc(
        "TRN2", target_bir_lowering=False, debug=False, num_devices=N_CORES
    )
    xm = nc.dram_tensor(
        "xm", [NCHUNK, 2, 128, KHALF * NB], mybir.dt.bfloat16, kind="ExternalInput"
    ).ap()
    xr = nc.dram_tensor(
        "xr", [128, XR_GRP], mybir.dt.bfloat16, kind="ExternalInput"
    ).ap()
    cpk = nc.dram_tensor(
        "cpk", [128, CPK_BYTES], mybir.dt.uint8, kind="ExternalInput"
    ).ap()
    outt = nc.dram_tensor(
        "outt", [NCLS, BPC], mybir.dt.float32, kind="ExternalOutput"
    ).ap()

    relu = mybir.ActivationFunctionType.Relu
    bf = mybir.dt.bfloat16
    f32 = mybir.dt.float32

    ctx = contextlib.ExitStack()
    with ctx:
        CONST = ctx.enter_context(
            nc.sbuf_tensor("CONST", [128, CPK_BYTES], mybir.dt.uint8)
        )
        W1 = [CONST[:, 256 * c : 256 * (c + 1)].bitcast(bf) for c in range(KMAIN)]
        W1R = CONST[:, 1536:1792].bitcast(bf)
        W2 = CONST[:, 1792:1812].bitcast(bf)
        B1 = CONST[:, 1812:1816].bitcast(f32)
        XR = ctx.enter_context(nc.sbuf_tensor("XR", [128, XR_GRP], bf))
        xh = [
            ctx.enter_context(nc.sbuf_tensor(f"xh{i}", [128, KHALF, NB], bf))
            for i in range(NXBUF)
        ]
        h1 = [
            ctx.enter_context(nc.sbuf_tensor(f"h1_{i}", [128, NB], bf))
            for i in range(NH1)
        ]
        ob = [
            ctx.enter_context(nc.sbuf_tensor(f"ob{i}", [NCLS, NB], f32))
            for i in range(NOB)
        ]
        ps1 = [
            ctx.enter_context(nc.psum_tensor(f"ps1_{i}", [128, NB], f32))
            for i in range(NPS1)
        ]
        ps2 = [
            ctx.enter_context(nc.psum_tensor(f"ps2_{i}", [NCLS, 512], f32))
            for i in range(NPS2)
        ]

        s_cpk = ctx.enter_context(nc.semaphore("s_cpk"))
        s_xr = ctx.enter_context(nc.semaphore("s_xr"))
        s_c0 = [ctx.enter_context(nc.semaphore(f"s_c0_{j}")) for j in range(2)]
        s_xs = [ctx.enter_context(nc.semaphore(f"s_xs{i}")) for i in range(NXBUF)]
        s_os = [ctx.enter_context(nc.semaphore(f"s_os{i}")) for i in range(NOB)]
        s_l1 = ctx.enter_context(nc.semaphore("s_l1"))
        s_act = ctx.enter_context(nc.semaphore("s_act"))
        s_l2 = ctx.enter_context(nc.semaphore("s_l2"))
        s_cp = ctx.enter_context(nc.semaphore("s_cp"))

        xs_count = [0] * NXBUF
        xs_target = {}

        block = ctx.enter_context(nc.Block())

        @block.sync
        def _(sync):
            sync.dma_start(CONST[:], cpk[:]).then_inc(s_cpk, 16)
            for h in range(2):
                sync.dma_start(
                    xh[h][:],
                    xm[0, h].rearrange("p (c b) -> p c b", c=KHALF),
                ).then_inc(s_c0[h], 16)
            sync.dma_start(XR[:], xr[:]).then_inc(s_xr, 16)
            for h in range(2, 2 * NCHUNK):
                if h >= NXBUF:
                    # slot h%NXBUF was last read by chunk (h-NXBUF)//2's L1
                    sync.wait_ge(s_l1, (h - NXBUF) // 2 + 1)
                sync.dma_start(
                    xh[h % NXBUF][:],
                    xm[h // 2, h % 2].rearrange("p (c b) -> p c b", c=KHALF),
                ).then_inc(s_xs[h % NXBUF], 16)
                xs_count[h % NXBUF] += 1
                xs_target[h] = 16 * xs_count[h % NXBUF]

        def emit_l1(tensor, n):
            if n >= NPS1:
                tensor.wait_ge(s_act, n - NPS1 + 1)
            p1 = ps1[n % NPS1]
            if n == 0:
                tensor.wait_ge(s_cpk, 16)
                for half in range(2):
                    tensor.wait_ge(s_c0[half], 16)
                    for c in range(half * KHALF, (half + 1) * KHALF):
                        for s in range(NSUB):
                            ssl = slice(s * 512, (s + 1) * 512)
                            nc.tensor.matmul(
                                p1[:, ssl],
                                W1[c],
                                xh[half][:, c % KHALF, ssl],
                                start=(c == 0),
                                stop=False,
                            )
            else:
                tensor.wait_ge(s_xs[(2 * n) % NXBUF], xs_target[2 * n])
                for c in range(KHALF):
                    for s in range(NSUB):
                        ssl = slice(s * 512, (s + 1) * 512)
                        nc.tensor.matmul(
                            p1[:, ssl],
                            W1[c],
                            xh[(2 * n) % NXBUF][:, c, ssl],
                            start=(c == 0),
                            stop=False,
                        )
                tensor.wait_ge(s_xs[(2 * n + 1) % NXBUF], xs_target[2 * n + 1])
                for c in range(KHALF, KMAIN):
                    for s in range(NSUB):
                        ssl = slice(s * 512, (s + 1) * 512)
                        nc.tensor.matmul(
                            p1[:, ssl],
                            W1[c],
                            xh[(2 * n + 1) % NXBUF][:, c - KHALF, ssl],
                            start=False,
                            stop=False,
                        )
            if n == 0:
                tensor.wait_ge(s_xr, 16)
            last = None
            for s in range(NSUB):
                ssl = slice(s * 512, (s + 1) * 512)
                boff = n * NB + s * 512
                g, coff = divmod(boff, XR_GRP)
                last = nc.tensor.matmul(
                    p1[:, ssl],
                    W1R[32 * g : 32 * g + KREM, :],
                    XR[32 * g : 32 * g + KREM, coff : coff + 512],
                    start=False,
                    stop=True,
                    tile_position=(32 * g, 0) if g == 3 else None,
                )
            last.then_inc(s_l1, 1)

        def emit_l2(tensor, n):
            tensor.wait_ge(s_act, n + 1)
            for s in range(NSUB):
                idx = 2 * n + s
                if idx >= NPS2:
                    tensor.wait_ge(s_cp, idx - NPS2 + 1)
                ssl = slice(s * 512, (s + 1) * 512)
                nc.tensor.matmul(
                    ps2[idx % NPS2][:],
                    W2[:],
                    h1[n % NH1][:, ssl],
                    start=True,
                    stop=True,
                ).then_inc(s_l2, 1)

        @block.tensor
        def _(tensor):
            emit_l1(tensor, 0)
            for n in range(1, NCHUNK):
                emit_l1(tensor, n)
                emit_l2(tensor, n - 1)
            emit_l2(tensor, NCHUNK - 1)

        @block.scalar
        def _(scalar):
            scalar.wait_ge(s_cpk, 16)
            for n in range(NCHUNK):
                scalar.wait_ge(s_l1, n + 1)
                if n >= NH1:
                    scalar.wait_ge(s_l2, 2 * (n - NH1) + 2)
                nc.scalar.activation(
                    h1[n % NH1][:], ps1[n % NPS1][:], relu, bias=B1[:]
                ).then_inc(s_act, 1)
                if n >= 2:
                    scalar.wait_ge(s_cp, 2 * (n - 1))
                    scalar.dma_start(
                        outt[:, (n - 2) * NB : (n - 1) * NB],
                        ob[(n - 2) % NOB][:],
                    ).then_inc(s_os[(n - 2) % NOB], 16)
            for n in (NCHUNK - 2, NCHUNK - 1):
                scalar.wait_ge(s_cp, 2 * (n + 1))
                scalar.dma_start(
                    outt[:, n * NB : (n + 1) * NB], ob[n % NOB][:]
                ).then_inc(s_os[n % NOB], 16)

        @block.vector
        def _(vector):
            for n in range(NCHUNK):
                for s in range(NSUB):
                    idx = 2 * n + s
                    vector.wait_ge(s_l2, idx + 1)
                    if s == 0 and n >= NOB:
                        vector.wait_ge(s_os[n % NOB], 16 * (n // NOB))
                    ssl = slice(s * 512, (s + 1) * 512)
                    nc.vector.tensor_copy(
                        ob[n % NOB][:, ssl], ps2[idx % NPS2][:]
                    ).then_inc(s_cp, 1)

    nc.compile()
    return nc


def _get_module():
    nc = _CACHE.get("nc")
    if nc is None:
        nc = _build_module()
        _CACHE["nc"] = nc
    return nc


def _prepare_inputs(x, conv_w, w1, b1, w2, b2):
    x = np.asarray(x, dtype=np.float32)
    conv_w = np.asarray(conv_w, dtype=np.float32)
    w1 = np.asarray(w1, dtype=np.float32)
    b1 = np.asarray(b1, dtype=np.float32)
    w2 = np.asarray(w2, dtype=np.float32)
    b2 = np.asarray(b2, dtype=np.float32)

    # Fold the 3x3 cross-correlation into w1: W1_eff[h, p, q] = sum over
    # (i, j, di, dj) with (p, q) == (i+di, j+dj) of w1[h, i*26+j]*conv_w.
    w1im = w1.reshape(HID, OUT_HW, OUT_HW)
    w1_eff = np.zeros((HID, IMG, IMG), np.float32)
    for di in range(KSZ):
        for dj in range(KSZ):
            w1_eff[:, di : di + OUT_HW, dj : dj + OUT_HW] += conv_w[di, dj] * w1im

    w1t_pad = np.zeros((FEAT, HPAD), _BF16)
    w1t_pad[:, :HID] = w1_eff.reshape(HID, FEAT).T.astype(_BF16)
    b1_pad = np.zeros(HPAD, np.float32)
    b1_pad[:HID] = b1
    b1_pad[HID] = 1.0  # h1 row 100 == relu(0+1) == 1: carries b2
    w2t_pad = np.zeros((HPAD, NCLS), _BF16)
    w2t_pad[:HID, :] = w2.T.astype(_BF16)
    w2t_pad[HID, :] = b2.astype(_BF16)

    # blocked W1: w1m[p, c*HPAD + m] = w1t_pad[c*128 + p, m]
    w1m_host = np.ascontiguousarray(
        w1t_pad[: KMAIN * 128].reshape(KMAIN, 128, HPAD).transpose(1, 0, 2)
    ).reshape(128, KMAIN * HPAD)
    # W1 remainder rows replicated at partition offsets 0/32/64/96
    w1r_host = np.zeros((128, HPAD), _BF16)
    for g in range(4):
        w1r_host[32 * g : 32 * g + KREM] = w1t_pad[KMAIN * 128 : FEAT]

    cpk = np.empty((128, CPK_BYTES), np.uint8)
    cpk[:, :1536] = w1m_host.view(np.uint8)
    cpk[:, 1536:1792] = w1r_host.view(np.uint8)
    cpk[:, 1792:1812] = w2t_pad.view(np.uint8)
    cpk[:, 1812:1816] = b1_pad.reshape(128, 1).view(np.uint8)

    xb = x.astype(_BF16)
    # xm[n, h, p, c*NB+b] = x[n*NB+b, (h*KHALF+c)*128+p]
    xcores = xb.reshape(N_CORES, NCHUNK, NB, FEAT)
    xm_all = np.ascontiguousarray(
        xcores[:, :, :, : KMAIN * 128]
        .reshape(N_CORES, NCHUNK, NB, 2, KHALF, 128)
        .transpose(0, 1, 3, 5, 4, 2)
    ).reshape(N_CORES, NCHUNK, 2, 128, KHALF * NB)
    # xr: batch groups of XR_GRP at partition offsets 32g..32g+KREM
    n_grp = BPC // XR_GRP
    xr_all = np.zeros((N_CORES, 128, XR_GRP), _BF16)
    rem = xb.reshape(N_CORES, BPC, FEAT)[:, :, KMAIN * 128 :]
    rem_g = rem.reshape(N_CORES, n_grp, XR_GRP, KREM).transpose(0, 1, 3, 2)
    for g in range(n_grp):
        xr_all[:, 32 * g : 32 * g + KREM, :] = rem_g[:, g]

    return [
        {"xm": xm_all[i], "xr": xr_all[i], "cpk": cpk} for i in range(N_CORES)
    ]


def _run_device(in_maps, trace=False, trace_cores=None):
    from concourse.bass_utils import run_bass_kernel_spmd

    nc = _get_module()
    return run_bass_kernel_spmd(
        nc,
        in_maps,
        core_ids=list(range(N_CORES)),
        trace=trace,
        trace_cores=trace_cores,
    )


def kernel(x, conv_w, w1, b1, w2, b2):
    in_maps = _prepare_inputs(x, conv_w, w1, b1, w2, b2)
    res = _run_device(in_maps)
    out = np.empty((B, NCLS), np.float32)
    for i in range(N_CORES):
        out[i * BPC : (i + 1) * BPC] = res.results[i]["outt"].T
    return out


# revision 10
# speedup vs baseline: 1.5392x; 1.0273x over previous
"""Trainium2 Bass kernel for DigitConvolutionalModel (self-contained).

Model: out = relu(conv3x3(x) @ w1.T + b1) @ w2.T + b2, x: [65536, 784] f32.

Algorithm
---------
The 3x3 valid cross-correlation is linear in x, so it is folded into the
first linear layer on the host (W1_eff[h] = conv-smeared w1[h]), giving a
plain 2-layer MLP:  out = relu(x @ W1_eff.T + b1) @ w2.T + b2.

Sharding: pure data parallelism — batch split 8 ways (8192 rows/core),
weights replicated; no collectives. Per core the kernel computes
out.T [10, 8192] with batch on the matmul free dim and features on
partitions; the host casts x to bf16 (fp32 accumulate in PSUM, measured
rel err ~3e-3) and lays it out in the exact blocked SBUF tile order
([chunk][partition = feat%128][feat chunk][batch]), so every device x DMA
is one fully contiguous transfer streaming through the Sync HWDGE FIFO at
~420 GB/s. The kernel is HBM-bandwidth-bound (12.6 MB of x per core);
TensorE work (~27 us warm) hides completely under the DMA stream.

Device pipeline (hand-written bacc, ~20 semaphores, no Tile scheduler):
  Sync   : consts + x half-chunk stream (strict FIFO, nothing else queued)
  Tensor : L1(0) L1(1) L2(0) L1(2) L2(1) ... L1(7) L2(6) L2(7)
           L1(n) = 12 K=128 matmuls + 2 K=16 remainder matmuls -> ps1 ring
           L2(n) = 2 matmuls h1 @ W2 -> ps2 ring
  Scalar : relu(ps1 + b1) -> h1 bf16, plus output DMAs (own HWDGE queue),
           lagged two chunks so they stay off the critical path
  Vector : ps2 -> ob f32 copies (PSUM cannot be DMA'd directly)

Tricks:
 - hidden dim padded 100 -> 128 with zero weight columns; b1_pad[100] = 1
   makes h1 row 100 == relu(0+1) == 1.0 and W2T row 100 = b2, folding the
   second-layer bias into the second matmul for free.
 - feature remainder (rows 768..783) handled by K=16 matmuls against a
   resident [128, 2048] tile holding batch groups at 32-aligned partition
   offsets (matmul base partitions must be 32-aligned; the 96 group needs
   an explicit tile_position).
 - all small constants (blocked W1, replicated W1 remainder, W2T+b2 rows,
   b1) are byte-packed into one [128, 1816] uint8 tensor: one contiguous
   DMA, no tiny-packet head-of-queue blocking; device uses bitcast views.
 - per-DMA-target semaphores with at most one outstanding DMA each
   (concurrent DMA slice completions interleave across queues, so shared
   counting semaphores would be racy).
"""

import sys

import numpy as np

if "/opt/trn_rl_repo" not in sys.path:
    sys.path.insert(0, "/opt/trn_rl_repo")

import ml_dtypes

B = 65536
IMG = 28
KSZ = 3
OUT_HW = IMG - KSZ + 1  # 26
FLAT = OUT_HW * OUT_HW  # 676
HID = 100
NCLS = 10
FEAT = IMG * IMG  # 784

N_CORES = 8
BPC = B // N_CORES  # 8192 batch rows per core
KMAIN = 6  # full 128-row feature chunks (768 rows)
KREM = FEAT - KMAIN * 128  # 16 remainder feature rows
HPAD = 128  # hidden dim padded 100 -> 128 (row 100 = bias carrier)
NB = 1024  # batch rows per chunk
NSUB = NB // 512  # 512-wide matmul subtiles per chunk
NCHUNK = BPC // NB  # 8
KHALF = KMAIN // 2  # k-blocks per half-chunk DMA
XR_GRP = 2048  # batch rows per 32-partition group in the XR tile

NXBUF = 8  # x half-chunk slot ring
NPS1 = 3  # ps1 ring (2 PSUM banks each)
NPS2 = 2  # ps2 ring (1 bank each)
NH1 = 3
NOB = 3
CPK_BYTES = 1816  # packed const bytes per partition

_BF16 = ml_dtypes.bfloat16
_CACHE = {}


def _build_module():
    import contextlib

    from concourse import bacc, mybir

    nc = bacc.Bacc(
        "TRN2", target_bir_lowering=False, debug=False, num_devices=N_CORES
    )
    xm = nc.dram_tensor(
        "xm", [NCHUNK, 2, 128, KHALF * NB], mybir.dt.bfloat16, kind="ExternalInput"
    ).ap()
    xr = nc.dram_tensor(
        "xr", [128, XR_GRP], mybir.dt.bfloat16, kind="ExternalInput"
    ).ap()
    cpk = nc.dram_tensor(
        "cpk", [128, CPK_BYTES], mybir.dt.uint8, kind="ExternalInput"
    ).ap()
    outt = nc.dram_tensor(
        "outt", [NCLS, BPC], mybir.dt.float32, kind="ExternalOutput"
    ).ap()

    relu = mybir.ActivationFunctionType.Relu
    bf = mybir.dt.bfloat16
    f32 = mybir.dt.float32

    ctx = contextlib.ExitStack()
    with ctx:
        CONST = ctx.enter_context(
            nc.sbuf_tensor("CONST", [128, CPK_BYTES], mybir.dt.uint8)
        )
        W1 = [CONST[:, 256 * c : 256 * (c + 1)].bitcast(bf) for c in range(KMAIN)]
        W1R = CONST[:, 1536:1792].bitcast(bf)
        W2 = CONST[:, 1792:1812].bitcast(bf)
        B1 = CONST[:, 1812:1816].bitcast(f32)
        XR = ctx.enter_context(nc.sbuf_tensor("XR", [128, XR_GRP], bf))
        xh = [
            ctx.enter_context(nc.sbuf_tensor(f"xh{i}", [128, KHALF, NB], bf))
            for i in range(NXBUF)
        ]
        h1 = [
            ctx.enter_context(nc.sbuf_tensor(f"h1_{i}", [128, NB], bf))
            for i in range(NH1)
        ]
        ob = [
            ctx.enter_context(nc.sbuf_tensor(f"ob{i}", [NCLS, NB], f32))
            for i in range(NOB)
        ]
        ps1 = [
            ctx.enter_context(nc.psum_tensor(f"ps1_{i}", [128, NB], f32))
            for i in range(NPS1)
        ]
        ps2 = [
            ctx.enter_context(nc.psum_tensor(f"ps2_{i}", [NCLS, 512], f32))
            for i in range(NPS2)
        ]

        s_cpk = ctx.enter_context(nc.semaphore("s_cpk"))
        s_xr = ctx.enter_context(nc.semaphore("s_xr"))
        s_c0 = [ctx.enter_context(nc.semaphore(f"s_c0_{j}")) for j in range(2)]
        s_xs = [ctx.enter_context(nc.semaphore(f"s_xs{i}")) for i in range(NXBUF)]
        s_os = [ctx.enter_context(nc.semaphore(f"s_os{i}")) for i in range(NOB)]
        s_l1 = ctx.enter_context(nc.semaphore("s_l1"))
        s_act = ctx.enter_context(nc.semaphore("s_act"))
        s_l2 = ctx.enter_context(nc.semaphore("s_l2"))
        s_cp = ctx.enter_context(nc.semaphore("s_cp"))

        xs_count = [0] * NXBUF
        xs_target = {}

        block = ctx.enter_context(nc.Block())

        @block.sync
        def _(sync):
            sync.dma_start(CONST[:], cpk[:]).then_inc(s_cpk, 16)
            for h in range(2):
                sync.dma_start(
                    xh[h][:],
                    xm[0, h].rearrange("p (c b) -> p c b", c=KHALF),
                ).then_inc(s_c0[h], 16)
            sync.dma_start(XR[:], xr[:]).then_inc(s_xr, 16)
            for h in range(2, 2 * NCHUNK):
                if h >= NXBUF:
                    # slot h%NXBUF was last read by chunk (h-NXBUF)//2's L1
                    sync.wait_ge(s_l1, (h - NXBUF) // 2 + 1)
                sync.dma_start(
                    xh[h % NXBUF][:],
                    xm[h // 2, h % 2].rearrange("p (c b) -> p c b", c=KHALF),
                ).then_inc(s_xs[h % NXBUF], 16)
                xs_count[h % NXBUF] += 1
                xs_target[h] = 16 * xs_count[h % NXBUF]

        def emit_l1(tensor, n):
            if n >= NPS1:
                tensor.wait_ge(s_act, n - NPS1 + 1)
            p1 = ps1[n % NPS1]
            if n == 0:
                tensor.wait_ge(s_cpk, 16)
                for half in range(2):
                    tensor.wait_ge(s_c0[half], 16)
                    for c in range(half * KHALF, (half + 1) * KHALF):
                        for s in range(NSUB):
                            ssl = slice(s * 512, (s + 1) * 512)
                            nc.tensor.matmul(
                                p1[:, ssl],
                                W1[c],
                                xh[half][:, c % KHALF, ssl],
                                start=(c == 0),
                                stop=False,
                            )
            else:
                tensor.wait_ge(s_xs[(2 * n) % NXBUF], xs_target[2 * n])
                for c in range(KHALF):
                    for s in range(NSUB):
                        ssl = slice(s * 512, (s + 1) * 512)
                        nc.tensor.matmul(
                            p1[:, ssl],
                            W1[c],
                            xh[(2 * n) % NXBUF][:, c, ssl],
                            start=(c == 0),
                            stop=False,
                        )
                tensor.wait_ge(s_xs[(2 * n + 1) % NXBUF], xs_target[2 * n + 1])
                for c in range(KHALF, KMAIN):
                    for s in range(NSUB):
                        ssl = slice(s * 512, (s + 1) * 512)
                        nc.tensor.matmul(
                            p1[:, ssl],
                            W1[c],
                            xh[(2 * n + 1) % NXBUF][:, c - KHALF, ssl],
                            start=False,
                            stop=False,
                        )
            if n == 0:
                tensor.wait_ge(s_xr, 16)
            last = None
            for s in range(NSUB):
                ssl = slice(s * 512, (s + 1) * 512)
                boff = n * NB + s * 512
                g, coff = divmod(boff, XR_GRP)
                last = nc.tensor.matmul(
                    p1[:, ssl],
                    W1R[32 * g : 32 * g + KREM, :],
                    XR[32 * g : 32 * g + KREM, coff : coff + 512],
                    start=False,
                    stop=True,
                    tile_position=(32 * g, 0) if g == 3 else None,
                )
            last.then_inc(s_l1, 1)

        def emit_l2(tensor, n):
            tensor.wait_ge(s_act, n + 1)
            for s in range(NSUB):
                idx = 2 * n + s
                if idx >= NPS2:
                    tensor.wait_ge(s_cp, idx - NPS2 + 1)
                ssl = slice(s * 512, (s + 1) * 512)
                nc.tensor.matmul(
                    ps2[idx % NPS2][:],
                    W2[:],
                    h1[n % NH1][:, ssl],
                    start=True,
                    stop=True,
                ).then_inc(s_l2, 1)

        @block.tensor
        def _(tensor):
            emit_l1(tensor, 0)
            for n in range(1, NCHUNK):
                emit_l1(tensor, n)
                emit_l2(tensor, n - 1)
            emit_l2(tensor, NCHUNK - 1)

        @block.scalar
        def _(scalar):
            scalar.wait_ge(s_cpk, 16)
            for n in range(NCHUNK):
                scalar.wait_ge(s_l1, n + 1)
                if n >= NH1:
                    scalar.wait_ge(s_l2, 2 * (n - NH1) + 2)
                nc.scalar.activation(
                    h1[n % NH1][:], ps1[n % NPS1][:], relu, bias=B1[:]
                ).then_inc(s_act, 1)
                if n >= 2:
                    scalar.wait_ge(s_cp, 2 * (n - 1))
                    scalar.dma_start(
                        outt[:, (n - 2) * NB : (n - 1) * NB],
                        ob[(n - 2) % NOB][:],
                    ).then_inc(s_os[(n - 2) % NOB], 16)
            for n in (NCHUNK - 2, NCHUNK - 1):
                scalar.wait_ge(s_cp, 2 * (n + 1))
                scalar.dma_start(
                    outt[:, n * NB : (n + 1) * NB], ob[n % NOB][:]
                ).then_inc(s_os[n % NOB], 16)

        @block.vector
        def _(vector):
            for n in range(NCHUNK):
                for s in range(NSUB):
                    idx = 2 * n + s
                    vector.wait_ge(s_l2, idx + 1)
                    if s == 0 and n >= NOB:
                        vector.wait_ge(s_os[n % NOB], 16 * (n // NOB))
                    ssl = slice(s * 512, (s + 1) * 512)
                    nc.vector.tensor_copy(
                        ob[n % NOB][:, ssl], ps2[idx % NPS2][:]
                    ).then_inc(s_cp, 1)

    nc.compile()
    return nc


def _get_module():
    nc = _CACHE.get("nc")
    if nc is None:
        nc = _build_module()
        _CACHE["nc"] = nc
    return nc


def _prepare_inputs(x, conv_w, w1, b1, w2, b2):
    x = np.asarray(x, dtype=np.float32)
    conv_w = np.asarray(conv_w, dtype=np.float32)
    w1 = np.asarray(w1, dtype=np.float32)
    b1 = np.asarray(b1, dtype=np.float32)
    w2 = np.asarray(w2, dtype=np.float32)
    b2 = np.asarray(b2, dtype=np.float32)

    # Fold the 3x3 cross-correlation into w1: W1_eff[h, p, q] = sum over
    # (i, j, di, dj) with (p, q) == (i+di, j+dj) of w1[h, i*26+j]*conv_w.
    w1im = w1.reshape(HID, OUT_HW, OUT_HW)
    w1_eff = np.zeros((HID, IMG, IMG), np.float32)
    for di in range(KSZ):
        for dj in range(KSZ):
            w1_eff[:, di : di + OUT_HW, dj : dj + OUT_HW] += conv_w[di, dj] * w1im

    w1t_pad = np.zeros((FEAT, HPAD), _BF16)
    w1t_pad[:, :HID] = w1_eff.reshape(HID, FEAT).T.astype(_BF16)
    b1_pad = np.zeros(HPAD, np.float32)
    b1_pad[:HID] = b1
    b1_pad[HID] = 1.0  # h1 row 100 == relu(0+1) == 1: carries b2
    w2t_pad = np.zeros((HPAD, NCLS), _BF16)
    w2t_pad[:HID, :] = w2.T.astype(_BF16)
    w2t_pad[HID, :] = b2.astype(_BF16)

    # blocked W1: w1m[p, c*HPAD + m] = w1t_pad[c*128 + p, m]
    w1m_host = np.ascontiguousarray(
        w1t_pad[: KMAIN * 128].reshape(KMAIN, 128, HPAD).transpose(1, 0, 2)
    ).reshape(128, KMAIN * HPAD)
    # W1 remainder rows replicated at partition offsets 0/32/64/96
    w1r_host = np.zeros((128, HPAD), _BF16)
    for g in range(4):
        w1r_host[32 * g : 32 * g + KREM] = w1t_pad[KMAIN * 128 : FEAT]

    cpk = np.empty((128, CPK_BYTES), np.uint8)
    cpk[:, :1536] = w1m_host.view(np.uint8)
    cpk[:, 1536:1792] = w1r_host.view(np.uint8)
    cpk[:, 1792:1812] = w2t_pad.view(np.uint8)
    cpk[:, 1812:1816] = b1_pad.reshape(128, 1).view(np.uint8)

    xb = x.astype(_BF16)
    # xm[n, h, p, c*NB+b] = x[n*NB+b, (h*KHALF+c)*128+p]
    xcores = xb.reshape(N_CORES, NCHUNK, NB, FEAT)
    xm_all = np.ascontiguousarray(
        xcores[:, :, :, : KMAIN * 128]
        .reshape(N_CORES, NCHUNK, NB, 2, KHALF, 128)
        .transpose(0, 1, 3, 5, 4, 2)
    ).reshape(N_CORES, NCHUNK, 2, 128, KHALF * NB)
    # xr: batch groups of XR_GRP at partition offsets 32g..32g+KREM
    n_grp = BPC // XR_GRP
    xr_all = np.zeros((N_CORES, 128, XR_GRP), _BF16)
    rem = xb.reshape(N_CORES, BPC, FEAT)[:, :, KMAIN * 128 :]
    rem_g = rem.reshape(N_CORES, n_grp, XR_GRP, KREM).transpose(0, 1, 3, 2)
    for g in range(n_grp):
        xr_all[:, 32 * g : 32 * g + KREM, :] = rem_g[:, g]

    return [
        {"xm": xm_all[i], "xr": xr_all[i], "cpk": cpk} for i in range(N_CORES)
    ]


def _ensure_accel_backend():
    # If the caller pinned JAX_PLATFORMS=cpu (common for running the jax
    # reference), the axon/neuron PJRT devices are invisible and the SPMD
    # run would fail; undo that for this process.
    import os

    import jax

    try:
        if all(d.platform == "cpu" for d in jax.devices()):
            if os.environ.get("JAX_PLATFORMS"):
                os.environ["JAX_PLATFORMS"] = ""
                from jax.extend import backend as _jeb

                _jeb.clear_backends()
    except Exception:
        pass


def _run_device(in_maps, trace=False, trace_cores=None):
    _ensure_accel_backend()
    from concourse.bass_utils import run_bass_kernel_spmd

    nc = _get_module()
    return run_bass_kernel_spmd(
        nc,
        in_maps,
        core_ids=list(range(N_CORES)),
        trace=trace,
        trace_cores=trace_cores,
    )


def kernel(x, conv_w, w1, b1, w2, b2):
    in_maps = _prepare_inputs(x, conv_w, w1, b1, w2, b2)
    res = _run_device(in_maps)
    out = np.empty((B, NCLS), np.float32)
    for i in range(N_CORES):
        out[i * BPC : (i + 1) * BPC] = res.results[i]["outt"].T
    return out
